# revision 13
# baseline (speedup 1.0000x reference)
"""Conv1d (B=32, C_in=C_out=64, L=16384, K=3, VALID) on 8 trn2 cores.

Strategy: data-parallel over batch (4 batches/core), polyphase-2 over L.
The host de-interleaves each batch's length axis into even/odd phases
stacked across 128 SBUF partitions: rows 0-63 = x[c, 0::2], rows
64-127 = x[c, 1::2].  The K=3 conv then needs only TWO PSUM-accumulated
matmuls per output chunk (vs 3 for the tap-per-matmul scheme):

  out_even(m) = w0 Xe[m] + w1 Xo[m] + w2 Xe[m+1]
  out_odd(m)  = w0 Xo[m] + w1 Xe[m+1] + w2 Xo[m+1]

  pass A: rhs = [Xe;Xo][:, m],   lhsT_A = [[w0^T, 0   ], [w1^T, w0^T]]
  pass B: rhs = [Xe;Xo][:, m+1], lhsT_B = [[w2^T, w1^T], [0,    w2^T]]

PSUM [128, n] = [out_even ch; out_odd ch]; the host re-interleaves.
This cuts TensorE busy ~48us -> ~30us, taking it off the critical path.

The kernel is fabric-bound: input + output (16.9 MB fp16) share the
~435 GB/s SBUF-AXI DMA fabric; the 16 SDMA engines round-robin at
packet granularity between the three DGE rings (sync-HWDGE,
scalar-HWDGE, gpsimd-SWDGE), and a ring sustains only ~286 GB/s by
itself (per-engine inter-packet gap).  So:
 - One input tile per batch [128, 8192], filled by sub-DMAs whose
   per-partition rows are 4 KB multiples (or a single <=4KB packet).
   A 4097-col halo transfer would split rows into 4096+4096+2-byte
   packets; the 2-byte runt packets cost a full packet slot each and
   cap the ring at ~190 GB/s (measured).  Chunk-col reads that span
   sub-DMA boundaries are handled by Tile's subtile deps.
 - Input sub-DMAs alternate the sync/scalar rings (two rings: input
   must finish as early as possible since it gates the whole tail).
 - All input triggers are issued (program order) before any output
   trigger on the sync/scalar rings, so an output trigger waiting on
   evacuation can never head-of-line-block input prefetch.
 - Early outputs drain on the gpsimd SWDGE ring; late outputs cycle
   over sync/scalar/gpsimd so the post-input drain uses all rings.
PSUM->SBUF evacuation (fused bias add, fp32->fp16) alternates whole
512-col chunks between ACT and DVE.  I/O is fp16 (~3e-4 rel err).
"""

import os

import numpy as np

from concourse import bacc, bass, mybir, tile
from concourse.bass_utils import run_bass_kernel_spmd

B, C, L, K = 32, 64, 16384, 3
LOUT = L - K + 1  # 16382
NCORES = 8
BPC = B // NCORES  # 4 batches per core
P = 128  # partitions (2 phases x C)
LH = L // 2  # 8192 phase-cols per batch
MOUT = LOUT // 2  # 8191 output phase-cols per batch
NJ = 512  # PSUM inner chunk (one fp32 bank)

F32 = mybir.dt.float32
F16 = mybir.dt.float16

IBUFS = int(os.environ.get("CONV_IBUFS", "4"))
OBUFS = int(os.environ.get("CONV_OBUFS", "8"))
WARMUP = int(os.environ.get("CONV_WARMUP", "8"))

# input sub-DMA col counts per batch (rows are 4KB-multiples / <=4KB)
IN_SUBS = {0: [1024, 1024, 2048, 4096]}
for _p in range(1, BPC):
    IN_SUBS[_p] = [4096, 4096]
# output chunk col counts per batch (first batch staged smaller so the
# output stream starts early; last batch tapers so the final chunks
# drain in parallel on two rings)
OUT_CHUNKS = {0: [2047, 2048, 4096], BPC - 1: [4095, 2048, 2048]}
for _p in range(1, BPC - 1):
    OUT_CHUNKS[_p] = [4095, 4096]
# output ring per global output-chunk index.  Early chunks go on the
# otherwise-idle SWDGE ring so the HWDGE rings stay pure-input (ring
# FIFO order then gives input strict priority); late chunks fan out
# over all three rings (sync/scalar carry outputs only after every
# input trigger is already queued there).
OUT_ENG = ["gpsimd", "gpsimd", "gpsimd",
           "sync", "scalar", "gpsimd", "sync", "scalar", "gpsimd", "sync"]

_NC_CACHE = []


def _build_nc():
    nc = bacc.Bacc("TRN2", target_bir_lowering=False, debug=False,
                   num_devices=NCORES)

    x2 = nc.dram_tensor("x2", [BPC, P, LH], F16, kind="ExternalInput")
    # w (pass A | pass B) and bias packed into one [128, 260] fp16
    # tensor: a 512B-row w DMA and a 4B-row bias DMA each trickle as
    # 128 small packets at ~300ns (measured ~2.5us per DMA) and must
    # not sit in a ring FIFO ahead of input data; one DMA on the idle
    # SWDGE ring hides the whole trickle.
    wb = nc.dram_tensor("wb", [P, 2 * P + 4], F16, kind="ExternalInput")
    y2 = nc.dram_tensor("y2", [BPC, P, MOUT], F16, kind="ExternalOutput")

    with tile.TileContext(nc) as tc:
        with (
            tc.tile_pool(name="const", bufs=1) as const_pool,
            tc.tile_pool(name="inp", bufs=IBUFS) as inp_pool,
            tc.tile_pool(name="outp", bufs=OBUFS) as outp_pool,
            tc.tile_pool(name="psum", bufs=8, space=bass.MemorySpace.PSUM)
            as psum_pool,
        ):
            isel = [0]

            def issue_in(p, first=0):
                it = inp_pool.tile([P, LH], F16, tag="in")
                c0 = 0
                for si, n in enumerate(IN_SUBS[p]):
                    if si >= first:
                        eng = nc.sync if isel[0] % 2 == 0 else nc.scalar
                        eng.dma_start(out=it[:, c0:c0 + n],
                                      in_=x2[p, :, c0:c0 + n])
                        isel[0] += 1
                    c0 += n
                return it

            # consts on the idle SWDGE ring; input sub-DMAs keep the
            # two HWDGE rings to themselves.
            wbt = const_pool.tile([P, 2 * P + 4], F16)
            nc.gpsimd.dma_start(out=wbt[:], in_=wb[:])
            wA = wbt[:, 0:P]
            wB = wbt[:, P:2 * P]
            # bias is fp32 stored in two f16 slots (DVE tensor_scalar
            # needs an fp32 scalar operand)
            bias = wbt[:, 2 * P:2 * P + 2].bitcast(F32)
            tiles = {0: issue_in(0), 1: issue_in(1)}

            # HAM warm-up: dummy matmuls on zeroed SBUF while the first
            # input DMA is in flight, so the PE clock gate is at 8/8
            # (2.4 GHz) when real work arrives.
            if WARMUP:
                wz = const_pool.tile([P, NJ], F16)
                nc.vector.memset(wz[:], 0.0)
                for i in range(WARMUP):
                    wp = psum_pool.tile([P, NJ], F32, tag="acc",
                                        name=f"warm{i}")
                    nc.tensor.matmul(wp[:], wz[:, :P], wz[:],
                                     start=True, stop=True)

            ci = 0  # global psum-chunk counter (ACT/DVE alternation)
            oi = 0  # global output-chunk index (ring assignment)
            for p in range(BPC):
                if p + 2 < BPC:
                    tiles[p + 2] = issue_in(p + 2)
                it = tiles.pop(p)
                m0 = 0
                for n in OUT_CHUNKS[p]:
                    ot = outp_pool.tile([P, 4096], F16, tag="out")
                    for j0 in range(m0, m0 + n, NJ):
                        nj = min(NJ, m0 + n - j0)
                        o0 = j0 - m0
                        pt = psum_pool.tile([P, NJ], F32, tag="acc")
                        nc.tensor.matmul(pt[:, :nj], wA,
                                         it[:, j0:j0 + nj],
                                         start=True, stop=False)
                        nc.tensor.matmul(pt[:, :nj], wB,
                                         it[:, j0 + 1:j0 + 1 + nj],
                                         start=False, stop=True)
                        # psum -> sbuf with fused bias add; whole chunk
                        # on one engine, alternating ACT/DVE
                        if ci % 2 == 0:
                            nc.scalar.add(ot[:, o0:o0 + nj], pt[:, :nj],
                                          add=bias)
                        else:
                            nc.vector.tensor_scalar_add(ot[:, o0:o0 + nj],
                                                        pt[:, :nj],
                                                        bias)
                        ci += 1
                    eng = {"sync": nc.sync, "scalar": nc.scalar,
                           "gpsimd": nc.gpsimd}[OUT_ENG[oi]]
                    eng.dma_start(out=y2[p, :, m0:m0 + n], in_=ot[:, :n])
                    oi += 1
                    m0 += n

    nc.compile()
    return nc


def _get_nc():
    if not _NC_CACHE:
        _NC_CACHE.append(_build_nc())
    return _NC_CACHE[0]


def _prep_weights(weight, bias):
    w = weight.astype(np.float32)
    wb = np.zeros((P, 2 * P + 4), np.float32)
    w0, w1, w2 = w[:, :, 0].T, w[:, :, 1].T, w[:, :, 2].T  # [C_in, C_out]
    wb[0:C, 0:C] = w0
    wb[C:P, 0:C] = w1
    wb[C:P, C:P] = w0
    wb[0:C, P:P + C] = w2
    wb[0:C, P + C:2 * P] = w1
    wb[C:P, P + C:2 * P] = w2
    wb16 = wb.astype(np.float16)
    # fp32 bias bit-packed into f16 slots 256:258
    wb16[:, 2 * P:2 * P + 2].view(np.float32)[:, 0] = np.concatenate(
        [bias, bias]).astype(np.float32)
    return wb16


def kernel(x, weight, bias, _want_results=False, **run_kwargs):
    x = np.asarray(x, np.float32)
    weight = np.asarray(weight, np.float32)
    bias = np.asarray(bias, np.float32)
    nc = _get_nc()
    wb = _prep_weights(weight, bias)

    # de-interleave length into even/odd phases stacked on partitions
    xh = x.astype(np.float16)
    in_maps = []
    for i in range(NCORES):
        xs = xh[BPC * i:BPC * (i + 1)]  # [BPC, C, L]
        xde = np.empty((BPC, P, LH), np.float16)
        xde[:, :C, :] = xs[:, :, 0::2]
        xde[:, C:, :] = xs[:, :, 1::2]
        in_maps.append({"x2": xde, "wb": wb})

    res = run_bass_kernel_spmd(nc, in_maps, list(range(NCORES)), **run_kwargs)

    out = np.empty((B, C, LOUT), np.float32)
    for i in range(NCORES):
        yde = res.results[i]["y2"]  # [BPC, P, MOUT] f16
        ob = out[BPC * i:BPC * (i + 1)]
        ob[:, :, 0::2] = yde[:, :C, :]
        ob[:, :, 1::2] = yde[:, C:, :]
    if _want_results:
        return out, res
    return out
